# revision 29
# baseline (speedup 1.0000x reference)
import sys
sys.path.insert(0, "/opt/trn_rl_repo")
"""CapsuleBlock kernel for TRN2, i-sharded across 8 cores.

Per-core (NI input capsules local):
  u = squash(x); 3 routing iterations where
    s[b,c,j]   = sum_i cw[b,c,i]*u_hat[b,c,i,j]   fused on PE (K=(i,d)),
    v = squash(s) after a 65KB AllReduce of s over the 8 cores,
    bl[b,c,i] += sum_j v[b,c,j]*u_hat[b,c,i,j]    via K=j T-matmuls + DVE dot.

Layouts: wn_bf [ip,(ic,c,j,d)] bf16; wjt DRAM [c,j,(d,i)] bf16;
u_*: [ip,(ic,d,b)]; bl/eb [ip,(ic,c,b)]; s/v [(cgl,b),(h,c4,j)] with
c = (h*4+cgl)*4 + c4; binc [(c4,b),(cg,i)] with cg = c//4.
"""

import os
import numpy as np
from contextlib import ExitStack

import concourse.bass as bass
import concourse.mybir as mybir
import concourse.tile as tile
from concourse import masks

f32 = mybir.dt.float32
bf16 = mybir.dt.bfloat16
fp8 = mybir.dt.float8e4
WJT_SCALE = 8.0
AX = mybir.AxisListType
OP = mybir.AluOpType
ACTF = mybir.ActivationFunctionType

B, C, J, D = 32, 32, 16, 16
ROUTINGS = 3
EPS = 1e-7
N_CORES = 8


def vt_off(c):
    """vt_rep free offset for capsule c: free layout (h, cgl, b)."""
    h, cgl = c // 16, (c // 4) % 4
    return (h * 4 + cgl) * B


def build_capsule_kernel(tc: tile.TileContext, v_out: bass.AP, x_in: bass.AP,
                         w_in: bass.AP, NI: int = 512):
    """v_out [B, C, J] f32; x_in [B, NI*D] f32; w_in [C, NI, J, D] f32."""
    nc = tc.nc
    IC = NI // 128
    CG = C // 4
    assert NI % 128 == 0

    ctx = ExitStack()
    main = ctx.enter_context(tc.tile_pool(name="main", bufs=1))
    psp = ctx.enter_context(tc.tile_pool(name="ps", bufs=2, space="PSUM"))
    dram = ctx.enter_context(tc.tile_pool(name="dram", bufs=1, space="DRAM"))

    # ---------- persistent SBUF (~97KB/partition) ----------
    wn_bf = main.tile([128, IC * C * J * D], bf16)      # [ip,(ic,c,j,d)]
    u_ipbf = main.tile([128, IC * D * B], bf16)         # [ip,(ic,d,b)]
    uz_bf = main.tile([128, IC * D * B], bf16)
    u_rep = main.tile([128, D * NI], bf16)              # [(c4,b),(d,i)]
    zsum = main.tile([128, IC * B], f32)
    zi = main.tile([128, IC * B], f32)
    sst = main.tile([128, CG * B], f32)                 # [(c4,jp),(cg,b)]
    s_sb = main.tile([128, CG * J], f32)                # [(cgl,b),(h,c4,j)]
    v_sb = main.tile([128, CG * J], f32)
    sq_sc = main.tile([128, 5 * CG], f32)
    vt_rep = main.tile([128, 2 * 4 * B], bf16)          # [(c4,jp),(h,cgl,b)]
    ident = main.tile([128, 128], f32)
    ident_bf = main.tile([128, 128], bf16)
    cst = main.tile([128, 2], f32)
    wjt_dram = dram.tile([C, 2 * J, D * NI], fp8)  # j-padded to 32 rows

    masks.make_identity(nc, ident[:])
    masks.make_identity(nc, ident_bf[:])
    nc.vector.memset(cst[:, 0:1], EPS)
    nc.vector.memset(cst[:, 1:2], 0.0)
    nc.vector.memset(sst[:], 0.0)   # pad rows (j=16..31 of each window) stay 0

    es1 = ExitStack()
    initp = es1.enter_context(tc.tile_pool(name="init", bufs=1))
    stage = es1.enter_context(tc.tile_pool(name="stage", bufs=2))
    if True:
        # ---------- load x; issue W chunk DMAs up front ----------
        x_sb = initp.tile([32, NI * D], f32, tag="xscratch")
        u_b = initp.tile([32, NI * D], f32)
        sqs = initp.tile([32, 4 * NI], f32)
        nc.sync.dma_start(x_sb[:], x_in[:])

        w_r = w_in.rearrange("c (ic ip) j d -> ic ip c (j d)", ip=128)
        qs = [nc.scalar, nc.gpsimd, nc.sync]
        wsts = []
        hn = C * J * D // 4
        for ic in range(IC):
            for hh in range(4):
                wst = stage.tile([128, hn], f32, tag="wst")
                wsts.append(wst)
                qs[(4 * ic + hh) % 3].dma_start(
                    wst[:],
                    w_r[ic][:, hh * (C // 4):(hh + 1) * (C // 4)])

        # ---------- squash -> u ----------
        nc.vector.tensor_mul(u_b[:], x_sb[:], x_sb[:])
        sq = sqs[:, 0:NI]
        nc.vector.tensor_reduce(sq, u_b[:].rearrange("b (i d) -> b i d", d=D),
                                axis=AX.X, op=OP.add)
        t1 = sqs[:, NI:2 * NI]
        nc.scalar.activation(t1, sq, ACTF.Sqrt, bias=cst[0:32, 0:1], scale=1.0)
        t2 = sqs[:, 2 * NI:3 * NI]
        nc.vector.tensor_scalar_add(t2, sq, 1.0)
        nc.vector.tensor_mul(t2, t2, t1)
        t3 = sqs[:, 3 * NI:4 * NI]
        nc.vector.reciprocal(t3, t2)
        nc.vector.tensor_mul(t3, t3, sq)
        nc.vector.tensor_tensor(
            u_b[:].rearrange("b (i d) -> b i d", d=D),
            x_sb[:].rearrange("b (i d) -> b i d", d=D),
            t3.rearrange("b (i one) -> b i one", one=1
                         ).broadcast_to((32, NI, D)), op=OP.mult)

        # u_dibf [32,(d,i)] bf16; replicate into u_rep over 4 col-groups
        u_dibf = initp.tile([32, D * NI], bf16, tag="xscratch")
        nc.vector.tensor_scalar(
            u_dibf[:].rearrange("b (d i) -> b d i", d=D),
            u_b[:].rearrange("b (i d) -> b d i", d=D),
            1.0 / WJT_SCALE, None, op0=OP.mult)
        for c4 in range(4):
            nc.sync.dma_start(u_rep[32 * c4:32 * (c4 + 1), :], u_dibf[:])

        # ---------- cast W chunks f32 -> bf16 ----------
        for ic in range(IC):
            for hh in range(4):
                off = ic * C * J * D + hh * hn
                dstq = wn_bf[:, off:off + hn]
                srcq = wsts[4 * ic + hh][:]
                if hh % 2:
                    nc.scalar.copy(dstq, srcq)
                else:
                    nc.vector.tensor_copy(dstq, srcq)

        # u_ipbf [ip,(ic,d,b)] via PE transposes of u_b
        u_bv = u_b[:].rearrange("b (ic ip d) -> b ip ic d", ip=128, d=D)
        for ic in range(IC):
            for dq in range(D // 4):
                ups = psp.tile([128, 2048], f32, tag="ps")
                for dd in range(4):
                    d = dq * 4 + dd
                    nc.tensor.transpose(ups[:, 32 * dd:32 * (dd + 1)],
                                        u_bv[:, :, ic, d],
                                        ident[0:32, 0:32])
                nc.vector.tensor_copy(
                    u_ipbf[:, (ic * D + dq * 4) * B:(ic * D + dq * 4 + 4) * B],
                    ups[:, 0:128])

    if True:
        # ---------- build wjt_dram ----------
        zpad = main.tile([16, D * NI], fp8)
        nc.vector.memset(zpad[:], 0.0)
        for c in range(C):
            [nc.sync, nc.scalar, nc.gpsimd][c % 3].dma_start(
                wjt_dram[c, J:2 * J, :], zpad[:])
        # W_jT build: PE transposes (wn.T @ I) + ACT/DVE cast to fp8
        for c in range(C):
            t1b = stage.tile([128, 2 * IC * 128], fp8, tag=f"t1_{c % 2}")
            for jh in range(2):
                for ic in range(IC):
                    src = wn_bf[:, (ic * C + c) * J * D + jh * 128:
                                (ic * C + c) * J * D + (jh + 1) * 128]
                    wps = psp.tile([128, 2048], f32, tag="ps")
                    nc.tensor.matmul(wps[:, 0:128], src, ident_bf[:],
                                     start=True, stop=True)
                    col = (jh * IC + ic) * 128
                    if ic % 2:
                        nc.scalar.activation(t1b[:, col:col + 128],
                                             wps[:, 0:128], ACTF.Copy,
                                             bias=0.0, scale=WJT_SCALE)
                    else:
                        nc.vector.tensor_scalar(
                            t1b[:, col:col + 128], wps[:, 0:128],
                            WJT_SCALE, None, op0=OP.mult)
            dst = wjt_dram[:, 0:J].rearrange(
                "c (jh j8) (d ic ip) -> c (j8 d) jh (ic ip)",
                jh=2, d=D, ip=128)[c]
            [nc.sync, nc.scalar, nc.gpsimd][c % 3].dma_start(
                dst, t1b[:].rearrange("p (jh f) -> p jh f", jh=2))
    es1.close()

    routp = ctx.enter_context(tc.tile_pool(name="rout", bufs=1))
    bl = routp.tile([128, IC * C * B], f32)             # [ip,(ic,c,b)]
    eb = routp.tile([128, IC * C * B], bf16)
    binc = routp.tile([128, CG * NI], f32)              # [(c4,b),(cg,i)]
    cupool = ctx.enter_context(tc.tile_pool(name="cu", bufs=1))
    wjtbuf = ctx.enter_context(tc.tile_pool(name="wjtbuf", bufs=1))
    prodp = ctx.enter_context(tc.tile_pool(name="prod", bufs=1))

    nc.vector.memset(bl[:], 0.0)
    wn_r = wn_bf[:].rearrange("p (ic c j d) -> p ic c j d", ic=IC, c=C, d=D)
    wn_dj = wn_bf[:].rearrange("p (ic c j d) -> p ic c d j", ic=IC, c=C, d=D)
    sst2 = routp.tile([128, C * J], f32)       # [(k,b),(c,j)] diag pieces
    ones_kb = routp.tile([128, B], f32)        # [(k,b), b'] = 4x I32
    for k in range(4):
        masks.make_identity(nc, ones_kb[32 * k:32 * (k + 1), :])

    def s_pass_k0():
        """s_0 = (1/C) sum_{i,d} u*W: c-shared moving -> M=128 stationaries.

        Per c-group g of 8 capsules: 64 accumulating matmuls with
        lhsT = wn[i, (c8, j)] (M=128), rhs = u_ipbf[i, (ic,d)-slice, b]
        (N=32). PSUM out [(c8l, j), b]. Then assemble into s_sb
        [(cgl,b), (h,c4,j)] via per-half transposes (partition moves).
        """
        for g in range(4):
            sps = psp.tile([128, 2048], f32, tag="ps")
            for ic in range(IC):
                for d in range(D):
                    first = (ic == 0 and d == 0)
                    last = (ic == IC - 1 and d == D - 1)
                    lhsT = wn_r[:, ic, g * 8:(g + 1) * 8, :, d]
                    rhs = u_ipbf[:, (ic * D + d) * B:(ic * D + d + 1) * B]
                    nc.tensor.matmul(sps[:, 0:B], lhsT, rhs,
                                     start=first, stop=last)
            # evac [128=(c8l,j), 32=b] f32, scaled by 1/C (same partitions)
            sev = main.tile([128, B], f32, name=f"sev{g}")
            nc.scalar.activation(sev[:], sps[:, 0:B],
                                 ACTF.Copy, bias=0.0, scale=1.0 / C)
            h = g // 2
            for hh in range(2):
                cgl = (g * 2 + hh) % 4
                # transpose [64=(c4,j), 32 b] -> [32 b, 64] at band 32*cgl
                # via regular matmul vs the matching identity block
                tps2 = psp.tile([128, 2048], f32, tag="ps")
                nc.tensor.matmul(tps2[32 * cgl:32 * (cgl + 1), 0:64],
                                 sev[64 * hh:64 * (hh + 1), :],
                                 ident[64 * hh:64 * (hh + 1),
                                       64 * hh:64 * (hh + 1)],
                                 start=True, stop=True,
                                 tile_position=(64 * hh, 32 * cgl))
                # cols of tps2 are (c4, j) -> s_sb free h*64+c4*16+j
                (nc.vector.tensor_copy if hh == 0 else nc.scalar.copy)(
                    s_sb[32 * cgl:32 * (cgl + 1), h * 64:(h + 1) * 64],
                    tps2[32 * cgl:32 * (cgl + 1), 0:64])

    def s_pass_rest(k):
        for cg in range(CG):
            sps = psp.tile([128, 2048], f32, tag="ps")
            cu = cupool.tile([128, IC * 4 * D * B], bf16, tag="cu")
            eng = nc.vector
            cuv = cu[:].rearrange("p (ic c4 d b) -> p ic c4 d b",
                                  ic=IC, c4=4, d=D)
            uzv = uz_bf[:].rearrange("p (one ic d b) -> p ic one d b",
                                     one=1, ic=IC, d=D
                                     ).broadcast_to((128, IC, 4, D, B))
            ebv = eb[:].rearrange("p (one ic c b) -> p ic c one b",
                                  one=1, ic=IC, c=C
                                  )[:, :, 4 * cg:4 * (cg + 1)
                                    ].broadcast_to((128, IC, 4, D, B))
            for ic in range(IC):
                eng.tensor_tensor(cuv[:, ic], uzv[:, ic], ebv[:, ic],
                                  op=OP.mult)
            sps = psp.tile([128, 2048], f32, tag="ps")
            for ic in range(IC):
                for d in range(D):
                    first = (ic == 0 and d == 0)
                    last = (ic == IC - 1 and d == D - 1)
                    for c4 in range(4):
                        c = cg * 4 + c4
                        lhsT = wn_r[:, ic, c, :, d]
                        rhs = cu[:, ((ic * 4 + c4) * D + d) * B:
                                 ((ic * 4 + c4) * D + d + 1) * B]
                        nc.tensor.matmul(sps[32 * c4:32 * c4 + 16, 0:B],
                                         lhsT, rhs, start=first, stop=last,
                                         tile_position=(0, 32 * c4))
            for c4 in range(4):
                dst = sst[32 * c4:32 * c4 + 16, cg * B:(cg + 1) * B]
                srcp = sps[32 * c4:32 * c4 + 16, 0:B]
                if c4 % 2:
                    nc.scalar.copy(dst, srcp)
                else:
                    nc.vector.tensor_copy(dst, srcp)
        # re-layout: sst [(c4,jp),(cg,b)] -> s_sb [(cgl,b),(h,c4,j)]
        for h in range(2):
            ssp = psp.tile([128, 2048], f32, tag="ps")
            nc.tensor.transpose(ssp[:, 0:128], sst[:, h * 128:(h + 1) * 128],
                                ident[:])
            nc.vector.tensor_copy(
                s_sb[:, h * 64:(h + 1) * 64].rearrange(
                    "p (c4 j) -> p c4 j", c4=4),
                ssp[:, 0:128].rearrange("p (c4 jp) -> p c4 jp", c4=4)[:, :, 0:J])

    def s_pass(k):
        if k == 0:
            s_pass_k0()
        else:
            s_pass_rest(k)
        ar_i = dram.tile([128, CG * J], bf16, tag=f"ari{k}")
        ar_o = dram.tile([128, CG * J], bf16, tag=f"aro{k}")
        s_bf = main.tile([128, CG * J], bf16, name="s_bf")
        nc.vector.tensor_copy(s_bf[:], s_sb[:])
        nc.sync.dma_start(ar_i[:], s_bf[:])
        if os.environ.get("NO_COLLECTIVE") == "1":
            nc.sync.dma_start(ar_o[:], ar_i[:])
        else:
            nc.gpsimd.collective_compute(
                "AllReduce", OP.add, replica_groups=[list(range(N_CORES))],
                ins=[ar_i.opt()], outs=[ar_o.opt()])
        nc.sync.dma_start(s_bf[:], ar_o[:])
        nc.scalar.copy(s_sb[:], s_bf[:])
        # squash -> v_sb
        nrm = sq_sc[:, 0:CG]
        s2b = binc[:, 0:CG * J]
        nc.vector.tensor_mul(s2b, s_sb[:], s_sb[:])
        nc.vector.tensor_reduce(nrm, s2b.rearrange("p (cg j) -> p cg j", j=J),
                                axis=AX.X, op=OP.add)
        st1 = sq_sc[:, CG:2 * CG]
        nc.scalar.activation(st1, nrm, ACTF.Sqrt, bias=cst[:, 0:1], scale=1.0)
        st2 = sq_sc[:, 2 * CG:3 * CG]
        nc.vector.tensor_scalar_add(st2, nrm, 1.0)
        nc.vector.tensor_mul(st2, st2, st1)
        st3 = sq_sc[:, 3 * CG:4 * CG]
        nc.vector.reciprocal(st3, st2)
        nc.vector.tensor_mul(st3, st3, nrm)
        nc.vector.tensor_tensor(
            v_sb[:].rearrange("p (cg j) -> p cg j", j=J),
            s_sb[:].rearrange("p (cg j) -> p cg j", j=J),
            st3.rearrange("p (cg one) -> p cg one", one=1
                          ).broadcast_to((128, CG, J)),
            op=OP.mult)

    def b_pass():
        # vT replicated into all 4 col windows: vt_rep[32*w+j, (h,cgl,b)]
        # window w holds the vT rows of capsules with c%4 == w.
        for h in range(2):
            for c4 in range(4):
                vps = psp.tile([128, 2048], f32, tag="ps")
                nc.tensor.matmul(
                    vps[32 * c4:32 * c4 + 16, 0:128],
                    v_sb[:, (h * 4 + c4) * J:(h * 4 + c4 + 1) * J], ident[:],
                    start=True, stop=True, tile_position=(0, 32 * c4))
                if (h * 4 + c4) % 2:
                    nc.vector.tensor_copy(
                        vt_rep[32 * c4:32 * c4 + 16, h * 128:(h + 1) * 128],
                        vps[32 * c4:32 * c4 + 16, 0:128])
                else:
                    nc.scalar.copy(
                        vt_rep[32 * c4:32 * c4 + 16, h * 128:(h + 1) * 128],
                        vps[32 * c4:32 * c4 + 16, 0:128])
        wjt_v = wjt_dram[:].rearrange("(cg c4) jp f -> cg (c4 jp) f", c4=4)
        for cg in range(CG):
            bslice = binc[:, cg * NI:(cg + 1) * NI]
            for dh in range(2):
                # one full-width DMA: rows 32*c4+j <- wjt[cg*4+c4, j, dh half]
                wb = wjtbuf.tile([128, (D // 2) * NI], fp8,
                                 tag=f"wb{dh}_{cg % 2}",
                                 name=f"wb_{dh}_{cg % 2}")
                (nc.scalar if dh == 0 else nc.gpsimd).dma_start(
                    wb[:], wjt_v[cg, :, dh * 8 * NI:(dh + 1) * 8 * NI])
                pr = prodp.tile([128, (D // 2) * NI], bf16,
                                tag=f"pr{cg % 2}", name=f"pr_{cg % 2}")
                tbf = prodp.tile([128, (D // 2) * NI], bf16,
                                 tag=f"tbf{cg % 2}", name=f"tbf_{cg % 2}")
                for dq in range(2):
                    tps = psp.tile([128, 2048], f32, tag="ps")
                    for dd in range(4):
                        d = dq * 4 + dd
                        for c4 in range(4):
                            c = cg * 4 + c4
                            nc.tensor.matmul(
                                tps[32 * c4:32 * (c4 + 1),
                                    NI * dd:NI * (dd + 1)],
                                vt_rep[32 * c4:32 * c4 + 16,
                                       vt_off(c):vt_off(c) + B],
                                wb[32 * c4:32 * c4 + 16, d * NI:(d + 1) * NI],
                                start=True, stop=True,
                                tile_position=(32 * c4, 32 * c4))
                    # evacuate PSUM on ACT (cast to bf16), multiply on DVE
                    if dq == 0:
                        nc.scalar.copy(tbf[:, 0:4 * NI], tps[:, 0:4 * NI])
                    else:
                        nc.vector.tensor_copy(tbf[:, 4 * NI:8 * NI],
                                              tps[:, 0:4 * NI])
                    nc.vector.tensor_tensor(
                        pr[:, dq * 4 * NI:(dq + 1) * 4 * NI],
                        tbf[:, dq * 4 * NI:(dq + 1) * 4 * NI],
                        u_rep[:, (dh * 8 + dq * 4) * NI:
                              (dh * 8 + dq * 4 + 4) * NI],
                        op=OP.mult)
                # sum over the 8 d's of this half (tree, in place)
                nc.gpsimd.tensor_add(pr[:, 0:4 * NI], pr[:, 0:4 * NI],
                                     pr[:, 4 * NI:8 * NI])
                nc.vector.tensor_add(pr[:, 0:2 * NI], pr[:, 0:2 * NI],
                                     pr[:, 2 * NI:4 * NI])
                nc.gpsimd.tensor_add(pr[:, 0:NI], pr[:, 0:NI], pr[:, NI:2 * NI])
                if dh == 0:
                    nc.gpsimd.tensor_copy(bslice, pr[:, 0:NI])
                else:
                    nc.vector.tensor_add(bslice, bslice, pr[:, 0:NI])
        # bl[ip,(ic,c,b)] += transpose(binc)
        for cg in range(CG):
            for ic in range(IC):
                bps = psp.tile([128, 2048], f32, tag="ps")
                nc.tensor.transpose(
                    bps[:, 0:128],
                    binc[:, cg * NI + ic * 128:cg * NI + (ic + 1) * 128],
                    ident[:])
                dst = bl[:, (ic * C + cg * 4) * B:(ic * C + cg * 4 + 4) * B]
                nc.vector.tensor_add(dst, dst, bps[:, 0:128])
        # softmax pieces: eb = exp(bl); zi = 1/sum_c eb; uz = u * zi
        nc.scalar.activation(eb[:], bl[:], ACTF.Exp, bias=cst[:, 1:2], scale=1.0)
        nc.vector.tensor_reduce(
            zsum[:], eb[:].rearrange("p (ic c b) -> p ic b c", c=C, b=B),
            axis=AX.X, op=OP.add)
        nc.vector.reciprocal(zi[:], zsum[:])
        nc.vector.tensor_tensor(
            uz_bf[:].rearrange("p (ic d b) -> p ic d b", ic=IC, d=D),
            u_ipbf[:].rearrange("p (ic d b) -> p ic d b", ic=IC, d=D),
            zi[:].rearrange("p (one ic b) -> p ic one b", one=1, ic=IC
                            ).broadcast_to((128, IC, D, B)),
            op=OP.mult)

    dbg = int(os.environ.get("CAPS_DEBUG_S0", "0"))
    if dbg:
        vd = v_out.rearrange("b c j -> (b c j)").rearrange("(p f) -> p f",
                                                           p=128)
        for k in range(dbg):
            s_pass(k)
            if k < dbg - 1:
                b_pass()
        nc.sync.dma_start(vd, s_sb[:])
        ctx.close()
        return

    for k in range(ROUTINGS):
        s_pass(k)
        if k < ROUTINGS - 1:
            b_pass()

    # v_sb [(cgl,b),(h,c4,j)] -> v_out [b, c, j], c = (h*4+cgl)*4+c4
    vo = v_out.rearrange("b (h cgl c4) j -> h cgl b (c4 j)", h=2, cgl=4)
    for h in range(2):
        nc.sync.dma_start(vo[h], v_sb[:, h * 64:(h + 1) * 64])
    ctx.close()


# ======================= runner =======================
import types
import concourse.bacc as bacc
from concourse import bass_utils


def _install_ntff_hook():
    """The agent image lacks antenv.axon_hooks; build it from the boot
    shim's ctypes NTFF driver so trace=True yields real HW profiles."""
    if "antenv.axon_hooks" in sys.modules:
        return
    try:
        sys.path.insert(0, "/root/.axon_site")
        from trn_agent_boot.trn_boot import _ntff_profile_via_ctypes
        hook = _ntff_profile_via_ctypes("/opt/axon/libaxon_pjrt.so")
        if hook is None:
            return
        m = types.ModuleType("antenv.axon_hooks")
        m.get_axon_ntff_profile_hook = lambda: hook
        m.set_axon_ntff_profile_hook = lambda h: None
        sys.modules["antenv.axon_hooks"] = m
    except Exception:
        pass

NI_TOT = 4096
NI_CORE = NI_TOT // N_CORES
_CACHE = {}


def _build():
    if "nc" in _CACHE:
        return _CACHE["nc"]
    nc = bacc.Bacc("TRN2", target_bir_lowering=False, debug=False,
                   enable_asserts=False, num_devices=N_CORES)
    x_d = nc.dram_tensor("x", (B, NI_CORE * D), f32, kind="ExternalInput").ap()
    w_d = nc.dram_tensor("W", (C, NI_CORE, J, D), f32, kind="ExternalInput").ap()
    v_d = nc.dram_tensor("v", (B, C, J), f32, kind="ExternalOutput").ap()
    with tile.TileContext(nc) as tc:
        build_capsule_kernel(tc, v_d, x_d, w_d, NI=NI_CORE)
    nc.compile()
    _CACHE["nc"] = nc
    return nc


def kernel(x: np.ndarray, W: np.ndarray) -> np.ndarray:
    x = np.ascontiguousarray(x, dtype=np.float32)
    W = np.ascontiguousarray(W, dtype=np.float32)
    nc = _build()
    in_maps = []
    for k in range(N_CORES):
        in_maps.append({
            "x": np.ascontiguousarray(x[:, k * NI_CORE * D:(k + 1) * NI_CORE * D]),
            "W": np.ascontiguousarray(W[:, k * NI_CORE:(k + 1) * NI_CORE]),
        })
    do_trace = os.environ.get("CAPS_TRACE", "0") == "1"
    if do_trace:
        _install_ntff_hook()
    res = bass_utils.run_bass_kernel_spmd(
        nc, in_maps, core_ids=list(range(N_CORES)), trace=do_trace,
        tmpdir=os.environ.get("CAPS_TRACE_DIR") or None)
    if res.exec_time_ns is not None:
        print(f"HW exec time: {res.exec_time_ns} ns")
    return res.results[0]["v"]



# revision 30
# speedup vs baseline: 1.0020x; 1.0020x over previous
import sys
sys.path.insert(0, "/opt/trn_rl_repo")
"""CapsuleBlock kernel for TRN2, i-sharded across 8 cores.

Per-core (NI input capsules local):
  u = squash(x); 3 routing iterations where
    s[b,c,j]   = sum_i cw[b,c,i]*u_hat[b,c,i,j]   fused on PE (K=(i,d)),
    v = squash(s) after a 65KB AllReduce of s over the 8 cores,
    bl[b,c,i] += sum_j v[b,c,j]*u_hat[b,c,i,j]    via K=j T-matmuls + DVE dot.

Layouts: wn_bf [ip,(ic,c,j,d)] bf16; wjt DRAM [c,j,(d,i)] bf16;
u_*: [ip,(ic,d,b)]; bl/eb [ip,(ic,c,b)]; s/v [(cgl,b),(h,c4,j)] with
c = (h*4+cgl)*4 + c4; binc [(c4,b),(cg,i)] with cg = c//4.
"""

import os
import numpy as np
from contextlib import ExitStack

import concourse.bass as bass
import concourse.mybir as mybir
import concourse.tile as tile
from concourse import masks

f32 = mybir.dt.float32
bf16 = mybir.dt.bfloat16
fp8 = mybir.dt.float8e4
WJT_SCALE = 8.0
AX = mybir.AxisListType
OP = mybir.AluOpType
ACTF = mybir.ActivationFunctionType

B, C, J, D = 32, 32, 16, 16
ROUTINGS = 3
EPS = 1e-7
N_CORES = 8


def vt_off(c):
    """vt_rep free offset for capsule c: free layout (h, cgl, b)."""
    h, cgl = c // 16, (c // 4) % 4
    return (h * 4 + cgl) * B


def build_capsule_kernel(tc: tile.TileContext, v_out: bass.AP, x_in: bass.AP,
                         w_in: bass.AP, NI: int = 512):
    """v_out [B, C, J] f32; x_in [B, NI*D] f32; w_in [C, NI, J, D] f32."""
    nc = tc.nc
    IC = NI // 128
    CG = C // 4
    assert NI % 128 == 0

    ctx = ExitStack()
    main = ctx.enter_context(tc.tile_pool(name="main", bufs=1))
    psp = ctx.enter_context(tc.tile_pool(name="ps", bufs=2, space="PSUM"))
    dram = ctx.enter_context(tc.tile_pool(name="dram", bufs=1, space="DRAM"))

    # ---------- persistent SBUF (~97KB/partition) ----------
    wn_bf = main.tile([128, IC * C * J * D], bf16)      # [ip,(ic,c,j,d)]
    u_ipbf = main.tile([128, IC * D * B], bf16)         # [ip,(ic,d,b)]
    uz_bf = main.tile([128, IC * D * B], bf16)
    u_rep = main.tile([128, D * NI], bf16)              # [(c4,b),(d,i)]
    zsum = main.tile([128, IC * B], f32)
    zi = main.tile([128, IC * B], f32)
    sst = main.tile([128, CG * B], f32)                 # [(c4,jp),(cg,b)]
    s_sb = main.tile([128, CG * J], f32)                # [(cgl,b),(h,c4,j)]
    v_sb = main.tile([128, CG * J], f32)
    sq_sc = main.tile([128, 5 * CG], f32)
    vt_rep = main.tile([128, 2 * 4 * B], bf16)          # [(c4,jp),(h,cgl,b)]
    ident = main.tile([128, 128], f32)
    ident_bf = main.tile([128, 128], bf16)
    cst = main.tile([128, 2], f32)
    wjt_dram = dram.tile([C, 2 * J, D * NI], fp8)  # j-padded to 32 rows

    masks.make_identity(nc, ident[:])
    masks.make_identity(nc, ident_bf[:])
    nc.vector.memset(cst[:, 0:1], EPS)
    nc.vector.memset(cst[:, 1:2], 0.0)
    nc.vector.memset(sst[:], 0.0)   # pad rows (j=16..31 of each window) stay 0

    es1 = ExitStack()
    initp = es1.enter_context(tc.tile_pool(name="init", bufs=1))
    stage = es1.enter_context(tc.tile_pool(name="stage", bufs=2))
    if True:
        # ---------- load x; issue W chunk DMAs up front ----------
        x_sb = initp.tile([32, NI * D], f32, tag="xscratch")
        u_b = initp.tile([32, NI * D], f32)
        sqs = initp.tile([32, 4 * NI], f32)
        nc.sync.dma_start(x_sb[:], x_in[:])

        w_r = w_in.rearrange("c (ic ip) j d -> ic ip c (j d)", ip=128)
        qs = [nc.scalar, nc.gpsimd, nc.sync]
        wsts = []
        hn = C * J * D // 4
        for ic in range(IC):
            for hh in range(4):
                wst = stage.tile([128, hn], f32, tag="wst")
                wsts.append(wst)
                qs[(4 * ic + hh) % 3].dma_start(
                    wst[:],
                    w_r[ic][:, hh * (C // 4):(hh + 1) * (C // 4)])

        # ---------- squash -> u ----------
        nc.vector.tensor_mul(u_b[:], x_sb[:], x_sb[:])
        sq = sqs[:, 0:NI]
        nc.vector.tensor_reduce(sq, u_b[:].rearrange("b (i d) -> b i d", d=D),
                                axis=AX.X, op=OP.add)
        t1 = sqs[:, NI:2 * NI]
        nc.scalar.activation(t1, sq, ACTF.Sqrt, bias=cst[0:32, 0:1], scale=1.0)
        t2 = sqs[:, 2 * NI:3 * NI]
        nc.vector.tensor_scalar_add(t2, sq, 1.0)
        nc.vector.tensor_mul(t2, t2, t1)
        t3 = sqs[:, 3 * NI:4 * NI]
        nc.vector.reciprocal(t3, t2)
        nc.vector.tensor_mul(t3, t3, sq)
        nc.vector.tensor_tensor(
            u_b[:].rearrange("b (i d) -> b i d", d=D),
            x_sb[:].rearrange("b (i d) -> b i d", d=D),
            t3.rearrange("b (i one) -> b i one", one=1
                         ).broadcast_to((32, NI, D)), op=OP.mult)

        # u_dibf [32,(d,i)] bf16; replicate into u_rep over 4 col-groups
        u_dibf = initp.tile([32, D * NI], bf16, tag="xscratch")
        nc.vector.tensor_scalar(
            u_dibf[:].rearrange("b (d i) -> b d i", d=D),
            u_b[:].rearrange("b (i d) -> b d i", d=D),
            1.0 / WJT_SCALE, None, op0=OP.mult)
        for c4 in range(4):
            nc.sync.dma_start(u_rep[32 * c4:32 * (c4 + 1), :], u_dibf[:])

        # ---------- cast W chunks f32 -> bf16 ----------
        for ic in range(IC):
            for hh in range(4):
                off = ic * C * J * D + hh * hn
                dstq = wn_bf[:, off:off + hn]
                srcq = wsts[4 * ic + hh][:]
                if hh % 2:
                    nc.scalar.copy(dstq, srcq)
                else:
                    nc.vector.tensor_copy(dstq, srcq)

        # u_ipbf [ip,(ic,d,b)] via PE transposes of u_b
        u_bv = u_b[:].rearrange("b (ic ip d) -> b ip ic d", ip=128, d=D)
        for ic in range(IC):
            for dq in range(D // 4):
                ups = psp.tile([128, 2048], f32, tag="ps")
                for dd in range(4):
                    d = dq * 4 + dd
                    nc.tensor.transpose(ups[:, 32 * dd:32 * (dd + 1)],
                                        u_bv[:, :, ic, d],
                                        ident[0:32, 0:32])
                nc.vector.tensor_copy(
                    u_ipbf[:, (ic * D + dq * 4) * B:(ic * D + dq * 4 + 4) * B],
                    ups[:, 0:128])

    if True:
        # ---------- build wjt_dram ----------
        zpad = main.tile([16, D * NI], fp8)
        nc.vector.memset(zpad[:], 0.0)
        for c in range(C):
            [nc.sync, nc.scalar, nc.gpsimd][c % 3].dma_start(
                wjt_dram[c, J:2 * J, :], zpad[:])
        # W_jT build: PE transposes (wn.T @ I) + ACT/DVE cast to fp8
        for c in range(C):
            t1b = stage.tile([128, 2 * IC * 128], fp8, tag=f"t1_{c % 2}")
            for jh in range(2):
                for ic in range(IC):
                    src = wn_bf[:, (ic * C + c) * J * D + jh * 128:
                                (ic * C + c) * J * D + (jh + 1) * 128]
                    wps = psp.tile([128, 2048], f32, tag="ps")
                    nc.tensor.matmul(wps[:, 0:128], src, ident_bf[:],
                                     start=True, stop=True)
                    col = (jh * IC + ic) * 128
                    if ic % 2:
                        nc.scalar.activation(t1b[:, col:col + 128],
                                             wps[:, 0:128], ACTF.Copy,
                                             bias=0.0, scale=WJT_SCALE)
                    else:
                        nc.vector.tensor_scalar(
                            t1b[:, col:col + 128], wps[:, 0:128],
                            WJT_SCALE, None, op0=OP.mult)
            dst = wjt_dram[:, 0:J].rearrange(
                "c (jh j8) (d ic ip) -> c (j8 d) jh (ic ip)",
                jh=2, d=D, ip=128)[c]
            [nc.sync, nc.scalar, nc.gpsimd][c % 3].dma_start(
                dst, t1b[:].rearrange("p (jh f) -> p jh f", jh=2))
    es1.close()

    routp = ctx.enter_context(tc.tile_pool(name="rout", bufs=1))
    bl = routp.tile([128, IC * C * B], f32)             # [ip,(ic,c,b)]
    eb = routp.tile([128, IC * C * B], bf16)
    binc = routp.tile([128, CG * NI], f32)              # [(c4,b),(cg,i)]
    cupool = ctx.enter_context(tc.tile_pool(name="cu", bufs=1))
    wjtbuf = ctx.enter_context(tc.tile_pool(name="wjtbuf", bufs=1))
    prodp = ctx.enter_context(tc.tile_pool(name="prod", bufs=1))

    nc.vector.memset(bl[:], 0.0)
    wn_r = wn_bf[:].rearrange("p (ic c j d) -> p ic c j d", ic=IC, c=C, d=D)
    wn_dj = wn_bf[:].rearrange("p (ic c j d) -> p ic c d j", ic=IC, c=C, d=D)
    sst2 = routp.tile([128, C * J], f32)       # [(k,b),(c,j)] diag pieces
    ones_kb = routp.tile([128, B], f32)        # [(k,b), b'] = 4x I32
    for k in range(4):
        masks.make_identity(nc, ones_kb[32 * k:32 * (k + 1), :])

    def s_pass_k0():
        """s_0 = (1/C) sum_{i,d} u*W: c-shared moving -> M=128 stationaries.

        Per c-group g of 8 capsules: 64 accumulating matmuls with
        lhsT = wn[i, (c8, j)] (M=128), rhs = u_ipbf[i, (ic,d)-slice, b]
        (N=32). PSUM out [(c8l, j), b]. Then assemble into s_sb
        [(cgl,b), (h,c4,j)] via per-half transposes (partition moves).
        """
        for g in range(4):
            sps = psp.tile([128, 2048], f32, tag="ps")
            for ic in range(IC):
                for d in range(D):
                    first = (ic == 0 and d == 0)
                    last = (ic == IC - 1 and d == D - 1)
                    lhsT = wn_r[:, ic, g * 8:(g + 1) * 8, :, d]
                    rhs = u_ipbf[:, (ic * D + d) * B:(ic * D + d + 1) * B]
                    nc.tensor.matmul(sps[:, 0:B], lhsT, rhs,
                                     start=first, stop=last)
            # evac [128=(c8l,j), 32=b] f32, scaled by 1/C (same partitions)
            sev = main.tile([128, B], f32, name=f"sev{g}")
            nc.scalar.activation(sev[:], sps[:, 0:B],
                                 ACTF.Copy, bias=0.0, scale=1.0 / C)
            h = g // 2
            for hh in range(2):
                cgl = (g * 2 + hh) % 4
                # transpose [64=(c4,j), 32 b] -> [32 b, 64] at band 32*cgl
                # via regular matmul vs the matching identity block
                tps2 = psp.tile([128, 2048], f32, tag="ps")
                nc.tensor.matmul(tps2[32 * cgl:32 * (cgl + 1), 0:64],
                                 sev[64 * hh:64 * (hh + 1), :],
                                 ident[64 * hh:64 * (hh + 1),
                                       64 * hh:64 * (hh + 1)],
                                 start=True, stop=True,
                                 tile_position=(64 * hh, 32 * cgl))
                # cols of tps2 are (c4, j) -> s_sb free h*64+c4*16+j
                (nc.vector.tensor_copy if hh == 0 else nc.scalar.copy)(
                    s_sb[32 * cgl:32 * (cgl + 1), h * 64:(h + 1) * 64],
                    tps2[32 * cgl:32 * (cgl + 1), 0:64])

    def s_pass_rest(k):
        for cg in range(CG):
            sps = psp.tile([128, 2048], f32, tag="ps")
            cu = cupool.tile([128, IC * 4 * D * B], bf16, tag="cu")
            eng = nc.vector
            cuv = cu[:].rearrange("p (ic c4 d b) -> p ic c4 d b",
                                  ic=IC, c4=4, d=D)
            uzv = uz_bf[:].rearrange("p (one ic d b) -> p ic one d b",
                                     one=1, ic=IC, d=D
                                     ).broadcast_to((128, IC, 4, D, B))
            ebv = eb[:].rearrange("p (one ic c b) -> p ic c one b",
                                  one=1, ic=IC, c=C
                                  )[:, :, 4 * cg:4 * (cg + 1)
                                    ].broadcast_to((128, IC, 4, D, B))
            for ic in range(IC):
                eng.tensor_tensor(cuv[:, ic], uzv[:, ic], ebv[:, ic],
                                  op=OP.mult)
            sps = psp.tile([128, 2048], f32, tag="ps")
            for ic in range(IC):
                for d in range(D):
                    first = (ic == 0 and d == 0)
                    last = (ic == IC - 1 and d == D - 1)
                    for c4 in range(4):
                        c = cg * 4 + c4
                        lhsT = wn_r[:, ic, c, :, d]
                        rhs = cu[:, ((ic * 4 + c4) * D + d) * B:
                                 ((ic * 4 + c4) * D + d + 1) * B]
                        nc.tensor.matmul(sps[32 * c4:32 * c4 + 16, 0:B],
                                         lhsT, rhs, start=first, stop=last,
                                         tile_position=(0, 32 * c4))
            for c4 in range(4):
                dst = sst[32 * c4:32 * c4 + 16, cg * B:(cg + 1) * B]
                srcp = sps[32 * c4:32 * c4 + 16, 0:B]
                if c4 % 2:
                    nc.scalar.copy(dst, srcp)
                else:
                    nc.vector.tensor_copy(dst, srcp)
        # re-layout: sst [(c4,jp),(cg,b)] -> s_sb [(cgl,b),(h,c4,j)]
        for h in range(2):
            ssp = psp.tile([128, 2048], f32, tag="ps")
            nc.tensor.transpose(ssp[:, 0:128], sst[:, h * 128:(h + 1) * 128],
                                ident[:])
            nc.vector.tensor_copy(
                s_sb[:, h * 64:(h + 1) * 64].rearrange(
                    "p (c4 j) -> p c4 j", c4=4),
                ssp[:, 0:128].rearrange("p (c4 jp) -> p c4 jp", c4=4)[:, :, 0:J])

    def s_pass(k):
        if k == 0:
            s_pass_k0()
        else:
            s_pass_rest(k)
        ar_i = dram.tile([128, CG * J], f32, tag=f"ari{k}")
        ar_o = dram.tile([128, CG * J], f32, tag=f"aro{k}")
        nc.sync.dma_start(ar_i[:], s_sb[:])
        if os.environ.get("NO_COLLECTIVE") == "1":
            nc.sync.dma_start(ar_o[:], ar_i[:])
        else:
            nc.gpsimd.collective_compute(
                "AllReduce", OP.add, replica_groups=[list(range(N_CORES))],
                ins=[ar_i.opt()], outs=[ar_o.opt()])
        nc.sync.dma_start(s_sb[:], ar_o[:])
        # squash -> v_sb
        nrm = sq_sc[:, 0:CG]
        s2b = binc[:, 0:CG * J]
        nc.vector.tensor_mul(s2b, s_sb[:], s_sb[:])
        nc.vector.tensor_reduce(nrm, s2b.rearrange("p (cg j) -> p cg j", j=J),
                                axis=AX.X, op=OP.add)
        st1 = sq_sc[:, CG:2 * CG]
        nc.scalar.activation(st1, nrm, ACTF.Sqrt, bias=cst[:, 0:1], scale=1.0)
        st2 = sq_sc[:, 2 * CG:3 * CG]
        nc.vector.tensor_scalar_add(st2, nrm, 1.0)
        nc.vector.tensor_mul(st2, st2, st1)
        st3 = sq_sc[:, 3 * CG:4 * CG]
        nc.vector.reciprocal(st3, st2)
        nc.vector.tensor_mul(st3, st3, nrm)
        nc.vector.tensor_tensor(
            v_sb[:].rearrange("p (cg j) -> p cg j", j=J),
            s_sb[:].rearrange("p (cg j) -> p cg j", j=J),
            st3.rearrange("p (cg one) -> p cg one", one=1
                          ).broadcast_to((128, CG, J)),
            op=OP.mult)

    def b_pass():
        # vT replicated into all 4 col windows: vt_rep[32*w+j, (h,cgl,b)]
        # window w holds the vT rows of capsules with c%4 == w.
        for h in range(2):
            for c4 in range(4):
                vps = psp.tile([128, 2048], f32, tag="ps")
                nc.tensor.matmul(
                    vps[32 * c4:32 * c4 + 16, 0:128],
                    v_sb[:, (h * 4 + c4) * J:(h * 4 + c4 + 1) * J], ident[:],
                    start=True, stop=True, tile_position=(0, 32 * c4))
                if (h * 4 + c4) % 2:
                    nc.vector.tensor_copy(
                        vt_rep[32 * c4:32 * c4 + 16, h * 128:(h + 1) * 128],
                        vps[32 * c4:32 * c4 + 16, 0:128])
                else:
                    nc.scalar.copy(
                        vt_rep[32 * c4:32 * c4 + 16, h * 128:(h + 1) * 128],
                        vps[32 * c4:32 * c4 + 16, 0:128])
        wjt_v = wjt_dram[:].rearrange("(cg c4) jp f -> cg (c4 jp) f", c4=4)
        for cg in range(CG):
            bslice = binc[:, cg * NI:(cg + 1) * NI]
            for dh in range(2):
                # one full-width DMA: rows 32*c4+j <- wjt[cg*4+c4, j, dh half]
                wb = wjtbuf.tile([128, (D // 2) * NI], fp8,
                                 tag=f"wb{dh}_{cg % 2}",
                                 name=f"wb_{dh}_{cg % 2}")
                (nc.scalar if dh == 0 else nc.gpsimd).dma_start(
                    wb[:], wjt_v[cg, :, dh * 8 * NI:(dh + 1) * 8 * NI])
                pr = prodp.tile([128, (D // 2) * NI], bf16,
                                tag=f"pr{cg % 2}", name=f"pr_{cg % 2}")
                tbf = prodp.tile([128, (D // 2) * NI], bf16,
                                 tag=f"tbf{cg % 2}", name=f"tbf_{cg % 2}")
                for dq in range(2):
                    tps = psp.tile([128, 2048], f32, tag="ps")
                    for dd in range(4):
                        d = dq * 4 + dd
                        for c4 in range(4):
                            c = cg * 4 + c4
                            nc.tensor.matmul(
                                tps[32 * c4:32 * (c4 + 1),
                                    NI * dd:NI * (dd + 1)],
                                vt_rep[32 * c4:32 * c4 + 16,
                                       vt_off(c):vt_off(c) + B],
                                wb[32 * c4:32 * c4 + 16, d * NI:(d + 1) * NI],
                                start=True, stop=True,
                                tile_position=(32 * c4, 32 * c4))
                    # evacuate PSUM on ACT (cast to bf16), multiply on DVE
                    if dq == 0:
                        nc.scalar.copy(tbf[:, 0:4 * NI], tps[:, 0:4 * NI])
                    else:
                        nc.vector.tensor_copy(tbf[:, 4 * NI:8 * NI],
                                              tps[:, 0:4 * NI])
                    nc.vector.tensor_tensor(
                        pr[:, dq * 4 * NI:(dq + 1) * 4 * NI],
                        tbf[:, dq * 4 * NI:(dq + 1) * 4 * NI],
                        u_rep[:, (dh * 8 + dq * 4) * NI:
                              (dh * 8 + dq * 4 + 4) * NI],
                        op=OP.mult)
                # sum over the 8 d's of this half (tree, in place)
                nc.gpsimd.tensor_add(pr[:, 0:4 * NI], pr[:, 0:4 * NI],
                                     pr[:, 4 * NI:8 * NI])
                nc.vector.tensor_add(pr[:, 0:2 * NI], pr[:, 0:2 * NI],
                                     pr[:, 2 * NI:4 * NI])
                nc.gpsimd.tensor_add(pr[:, 0:NI], pr[:, 0:NI], pr[:, NI:2 * NI])
                if dh == 0:
                    nc.gpsimd.tensor_copy(bslice, pr[:, 0:NI])
                else:
                    nc.vector.tensor_add(bslice, bslice, pr[:, 0:NI])
        # bl[ip,(ic,c,b)] += transpose(binc)
        for cg in range(CG):
            for ic in range(IC):
                bps = psp.tile([128, 2048], f32, tag="ps")
                nc.tensor.transpose(
                    bps[:, 0:128],
                    binc[:, cg * NI + ic * 128:cg * NI + (ic + 1) * 128],
                    ident[:])
                dst = bl[:, (ic * C + cg * 4) * B:(ic * C + cg * 4 + 4) * B]
                nc.vector.tensor_add(dst, dst, bps[:, 0:128])
        # softmax pieces: eb = exp(bl); zi = 1/sum_c eb; uz = u * zi
        nc.scalar.activation(eb[:], bl[:], ACTF.Exp, bias=cst[:, 1:2], scale=1.0)
        nc.vector.tensor_reduce(
            zsum[:], eb[:].rearrange("p (ic c b) -> p ic b c", c=C, b=B),
            axis=AX.X, op=OP.add)
        nc.vector.reciprocal(zi[:], zsum[:])
        nc.vector.tensor_tensor(
            uz_bf[:].rearrange("p (ic d b) -> p ic d b", ic=IC, d=D),
            u_ipbf[:].rearrange("p (ic d b) -> p ic d b", ic=IC, d=D),
            zi[:].rearrange("p (one ic b) -> p ic one b", one=1, ic=IC
                            ).broadcast_to((128, IC, D, B)),
            op=OP.mult)

    dbg = int(os.environ.get("CAPS_DEBUG_S0", "0"))
    if dbg:
        vd = v_out.rearrange("b c j -> (b c j)").rearrange("(p f) -> p f",
                                                           p=128)
        for k in range(dbg):
            s_pass(k)
            if k < dbg - 1:
                b_pass()
        nc.sync.dma_start(vd, s_sb[:])
        ctx.close()
        return

    for k in range(ROUTINGS):
        s_pass(k)
        if k < ROUTINGS - 1:
            b_pass()

    # v_sb [(cgl,b),(h,c4,j)] -> v_out [b, c, j], c = (h*4+cgl)*4+c4
    vo = v_out.rearrange("b (h cgl c4) j -> h cgl b (c4 j)", h=2, cgl=4)
    for h in range(2):
        nc.sync.dma_start(vo[h], v_sb[:, h * 64:(h + 1) * 64])
    ctx.close()


# ======================= runner =======================
import types
import concourse.bacc as bacc
from concourse import bass_utils


def _install_ntff_hook():
    """The agent image lacks antenv.axon_hooks; build it from the boot
    shim's ctypes NTFF driver so trace=True yields real HW profiles."""
    if "antenv.axon_hooks" in sys.modules:
        return
    try:
        sys.path.insert(0, "/root/.axon_site")
        from trn_agent_boot.trn_boot import _ntff_profile_via_ctypes
        hook = _ntff_profile_via_ctypes("/opt/axon/libaxon_pjrt.so")
        if hook is None:
            return
        m = types.ModuleType("antenv.axon_hooks")
        m.get_axon_ntff_profile_hook = lambda: hook
        m.set_axon_ntff_profile_hook = lambda h: None
        sys.modules["antenv.axon_hooks"] = m
    except Exception:
        pass

NI_TOT = 4096
NI_CORE = NI_TOT // N_CORES
_CACHE = {}


def _build():
    if "nc" in _CACHE:
        return _CACHE["nc"]
    nc = bacc.Bacc("TRN2", target_bir_lowering=False, debug=False,
                   enable_asserts=False, num_devices=N_CORES)
    x_d = nc.dram_tensor("x", (B, NI_CORE * D), f32, kind="ExternalInput").ap()
    w_d = nc.dram_tensor("W", (C, NI_CORE, J, D), f32, kind="ExternalInput").ap()
    v_d = nc.dram_tensor("v", (B, C, J), f32, kind="ExternalOutput").ap()
    with tile.TileContext(nc) as tc:
        build_capsule_kernel(tc, v_d, x_d, w_d, NI=NI_CORE)
    nc.compile()
    _CACHE["nc"] = nc
    return nc


def kernel(x: np.ndarray, W: np.ndarray) -> np.ndarray:
    x = np.ascontiguousarray(x, dtype=np.float32)
    W = np.ascontiguousarray(W, dtype=np.float32)
    nc = _build()
    in_maps = []
    for k in range(N_CORES):
        in_maps.append({
            "x": np.ascontiguousarray(x[:, k * NI_CORE * D:(k + 1) * NI_CORE * D]),
            "W": np.ascontiguousarray(W[:, k * NI_CORE:(k + 1) * NI_CORE]),
        })
    do_trace = os.environ.get("CAPS_TRACE", "0") == "1"
    if do_trace:
        _install_ntff_hook()
    res = bass_utils.run_bass_kernel_spmd(
        nc, in_maps, core_ids=list(range(N_CORES)), trace=do_trace,
        tmpdir=os.environ.get("CAPS_TRACE_DIR") or None)
    if res.exec_time_ns is not None:
        print(f"HW exec time: {res.exec_time_ns} ns")
    return res.results[0]["v"]

